# revision 1
# baseline (speedup 1.0000x reference)
"""Trainium2 Bass kernel for windowed multi-head attention with relative
position bias (Swin-style block):

    qkv = x @ qkv_w.T + [q_bias, 0, v_bias]
    q, k, v = split(qkv);  q *= hd**-0.5
    attn = softmax(q @ k.T + rel_table[rel_index])
    out  = (attn @ v) @ proj_w.T + proj_b

Shapes: x [8, 32, 32, 768], 12 heads, head_dim 64, N=1024 tokens.

Sharding: pure data-parallel — one batch element per NeuronCore, 8 cores,
no collectives. Each core runs an identical NEFF on its own slice.

Per-core dataflow (everything transposed so no on-chip transposes needed):
  xT [c,t] as lhsT, wv       -> V natural [t,o'] (fp16)     PE fp32r, K=128
  wqk as lhsT, xT as rhs     -> Q_T / zero-padded K^T (fp16)
  S^T[k,q] = K_pad(h)^T @ Q_T  (zero-pad selects the head, fp16, K=128)
  S^T += biasT  (DVE in-place on PSUM, fp16 bias DMA'd from DRAM)
  P^T = exp(S^T)  (ACT, PSUM -> fp16 SBUF; logits are O(1), no max pass)
  out2T[d,q] (+sums row via an appended ones column) = [V_h|1]^T @ P^T
  normalization deferred: unnormalized out2T -> attn_outT (f32r), row sums
  -> DRAM scratch; reciprocal prep for heads 0-9 runs early (after head 9)
  so only the last head-pair's inverse is on the tail critical path; inverse
  rows are broadcast across partitions with tiny K=2 selector matmuls on the
  PE and applied with one multiply per 128x512 tile, pipelined into the
  projection's contraction steps.
  y^T[co,t] = wproj^T @ attn_outT  (f32r), + proj_b on eviction.
Host reassembles y from per-core y^T.
"""

import numpy as np

_CACHE = {}

B = 8
WS = 32
N = WS * WS            # 1024 tokens
C = 768
NH = 12
HD = 64
P = 128
QC = 2                 # q chunks of 512
QN = N // QC           # 512
KT = N // P            # 8 k tiles
CT = C // P            # 6 contraction tiles
OT_QK = (2 * C) // P   # 12 output tiles for q,k rows
VC = 2                 # v output chunks of 384
VN = C // VC           # 384


def _build():
    import concourse.bass as bass
    import concourse.bacc as bacc
    import concourse.mybir as mybir
    import concourse.tile as tile
    from concourse.bass import _add_dep_helper

    f32 = mybir.dt.float32
    f32r = mybir.dt.float32r
    f16 = mybir.dt.float16
    AF = mybir.ActivationFunctionType

    nc = bacc.Bacc(None, target_bir_lowering=False)

    xT_d = nc.dram_tensor("xT", [C, N], f32, kind="ExternalInput")
    wqk_d = nc.dram_tensor("wqk", [C, 2 * C], f32, kind="ExternalInput")
    wv_d = nc.dram_tensor("wv", [C, C], f32, kind="ExternalInput")
    wproj_d = nc.dram_tensor("wproj", [C, C], f32, kind="ExternalInput")
    qkb_d = nc.dram_tensor("qkb", [OT_QK, P], f32, kind="ExternalInput")
    vb_d = nc.dram_tensor("vb", [C], f32, kind="ExternalInput")
    pb_d = nc.dram_tensor("pb", [CT, P], f32, kind="ExternalInput")
    biasT_d = nc.dram_tensor("biasT", [NH, N, N], f16, kind="ExternalInput")
    sel_d = nc.dram_tensor("sel", [2, P], f32, kind="ExternalInput")
    yT_d = nc.dram_tensor("yT", [C, N], f32, kind="ExternalOutput")
    sums_d = nc.dram_tensor("sums_scr", [NH * QC, QN], f32)
    inv_d = nc.dram_tensor("inv_scr", [NH * QC, QN], f32)

    with tile.TileContext(nc) as tc:
        with (
            tc.tile_pool(name="cst", bufs=1) as cst,
            tc.tile_pool(name="bias_pool", bufs=2) as bias_pool,
        ):
            # ---- permanent buffers ----
            q_t = cst.tile([P, CT, N], f16)        # Q^T [o, t]
            k_pad = cst.tile([P, NH, N], f16)      # zero-padded K^T per head
            v_aug = cst.tile([P, KT, NH, HD + 1], f16)  # V + ones column
            attn_outT = cst.tile([P, CT, N], f32r)
            qkb = cst.tile([P, OT_QK], f32)
            sel = cst.tile([2, P], f32)
            vb_bc = cst.tile([P, C], f32)
            pbias = cst.tile([P, CT], f32)

            # biasT tiles rotate through this pre-phase-1 pool (its address
            # space is disjoint from phase 1 -> DMAs don't wait on p1 release)
            biasT = {}

            def load_bias(h):
                biasT[h] = bias_pool.tile([P, KT, N], f16, tag="biasT",
                                          name=f"biasT{h}")
                nc.sync.dma_start(
                    biasT[h], biasT_d[h].rearrange("(kt p) q -> p kt q", p=P))

            nc.sync.dma_start(qkb, qkb_d[:].rearrange("j p -> p j"))
            nc.sync.dma_start(sel, sel_d[:])
            nc.sync.dma_start(
                vb_bc, bass.AP(tensor=vb_d, offset=0, ap=[[0, P], [1, C]]))
            nc.sync.dma_start(pbias, pb_d[:].rearrange("j p -> p j"))

            # zero the pad halves of k_pad, set ones columns of v_aug
            nc.vector.memset(k_pad[64:128, 0:NH:2, :], 0.0)
            nc.vector.memset(k_pad[0:64, 1:NH:2, :], 0.0)
            nc.vector.memset(v_aug[:, :, :, HD:HD + 1], 1.0)

            # ---- phase 1: qkv projections (fp32r), V part first ----
            with (
                tc.tile_pool(name="p1", bufs=1) as p1,
                tc.tile_pool(name="ps_1", bufs=2, space="PSUM") as ps_1,
                tc.tile_pool(name="ps_v", bufs=2, space="PSUM") as ps_v,
            ):
                xT = p1.tile([P, CT, N], f32r)
                wqk = p1.tile([P, CT, 2 * C], f32r)
                wv = p1.tile([P, CT, C], f32r)
                xT_src = xT_d[:].rearrange("(k p) t -> p k t", p=P).bitcast(f32r)
                wqk_src = wqk_d[:].rearrange("(k p) o -> p k o", p=P).bitcast(f32r)
                wv_src = wv_d[:].rearrange("(k p) o -> p k o", p=P).bitcast(f32r)
                nc.sync.dma_start(xT[:, 0, 0:P], xT_src[:, 0, 0:P])
                nc.sync.dma_start(wv[:, 0, :], wv_src[:, 0, :])
                nc.sync.dma_start(xT[:, 0, P:], xT_src[:, 0, P:])
                for k in range(1, CT):
                    nc.sync.dma_start(xT[:, k, :], xT_src[:, k, :])
                    nc.sync.dma_start(wv[:, k, :], wv_src[:, k, :])
                load_bias(0)
                load_bias(1)
                for k in range(CT):
                    nc.sync.dma_start(wqk[:, k, :], wqk_src[:, k, :])

                # V: out [t-tile, o'-chunk]; evict with v_bias broadcast add
                for tt in range(KT):
                    pvs = [ps_v.tile([P, VN], f32, tag=f"pv{vc}",
                                     name=f"pv{vc}") for vc in range(VC)]
                    for k in range(CT):
                        for vc in range(VC):
                            nc.tensor.matmul(
                                pvs[vc], xT[:, k, tt * P:(tt + 1) * P],
                                wv[:, k, vc * VN:(vc + 1) * VN],
                                start=(k == 0), stop=(k == CT - 1))
                    for vc in range(VC):
                        h0 = vc * (NH // VC)
                        nc.vector.tensor_add(
                            v_aug[:, tt, h0:h0 + NH // VC, 0:HD],
                            pvs[vc], vb_bc[:, vc * VN:(vc + 1) * VN])

                # Q^T and K^T: out [o-tile, t-chunk]
                for j in range(OT_QK):
                    pqs = [ps_1.tile([P, QN], f32, tag=f"pqkv{qc}",
                                     name=f"pq{qc}") for qc in range(QC)]
                    for k in range(CT):
                        for qc in range(QC):
                            nc.tensor.matmul(
                                pqs[qc], wqk[:, k, j * P:(j + 1) * P],
                                xT[:, k, qc * QN:(qc + 1) * QN],
                                start=(k == 0), stop=(k == CT - 1))
                    for qc in range(QC):
                        pq = pqs[qc]
                        if j < CT:
                            nc.scalar.activation(
                                q_t[:, j, qc * QN:(qc + 1) * QN], pq,
                                AF.Identity, bias=qkb[:, j:j + 1], scale=1.0)
                        else:
                            h0 = 2 * (j - CT)
                            nc.scalar.activation(
                                k_pad[0:64, h0, qc * QN:(qc + 1) * QN],
                                pq[0:64, :], AF.Identity,
                                bias=qkb[0:64, j:j + 1], scale=1.0)
                            nc.scalar.activation(
                                k_pad[64:128, h0 + 1, qc * QN:(qc + 1) * QN],
                                pq[64:128, :], AF.Identity,
                                bias=qkb[64:128, j:j + 1], scale=1.0)

            # ---- phase 2+3 pools (p1 closed, space reclaimed) ----
            with (
                tc.tile_pool(name="p2", bufs=1) as p2,
                tc.tile_pool(name="pt_pool", bufs=3) as pt_pool,
                tc.tile_pool(name="sums_pool", bufs=4) as sums_pool,
                tc.tile_pool(name="s2_pool", bufs=4) as s2_pool,
            ):
                wproj = p2.tile([P, CT, C], f32r)
                wproj_src = wproj_d[:].rearrange(
                    "(k p) o -> p k o", p=P).bitcast(f32r)
                for k in range(CT):
                    nc.sync.dma_start(wproj[:, k, :], wproj_src[:, k, :])

                srow_dmas = []
                inv_w = {}

                def norm_prep_dve(row_lo, row_hi, tag):
                    nrow = row_hi - row_lo
                    ssb = p2.tile([nrow, QN], f32, tag=f"ssb{tag}",
                                  name=f"ssb{tag}")
                    g = nc.sync.dma_start(ssb, sums_d[row_lo:row_hi])
                    for dep in srow_dmas[row_lo:row_hi]:
                        _add_dep_helper(g.ins, dep.ins, sync=True,
                                        reason="sums scratch RAW")
                    nc.vector.reciprocal(ssb, ssb)
                    inv_w[tag] = nc.sync.dma_start(inv_d[row_lo:row_hi], ssb)

                def norm_prep_act(row_lo, row_hi, tag):
                    # inv = exp(-ln(s)) on ACT (ln+exp share one table set)
                    nrow = row_hi - row_lo
                    ssb = p2.tile([nrow, QN], f32, tag=f"ssb{tag}",
                                  name=f"ssb{tag}")
                    g = nc.sync.dma_start(ssb, sums_d[row_lo:row_hi])
                    for dep in srow_dmas[row_lo:row_hi]:
                        _add_dep_helper(g.ins, dep.ins, sync=True,
                                        reason="sums scratch RAW")
                    nc.scalar.activation(ssb, ssb, AF.Ln, bias=0.0, scale=1.0)
                    nc.scalar.activation(ssb, ssb, AF.Exp, bias=0.0, scale=-1.0)
                    inv_w[tag] = nc.sync.dma_start(inv_d[row_lo:row_hi], ssb)

                def norm_apply(jlist, ps_n):
                    for j in jlist:
                        for qc in range(QC):
                            s2 = s2_pool.tile([2, QN], f32, tag="s2",
                                              name="s2")
                            r = nc.sync.dma_start(
                                s2,
                                bass.AP(tensor=inv_d,
                                        offset=(4 * j + qc) * QN,
                                        ap=[[2 * QN, 2], [1, QN]]))
                            wtag = "a" if j < 5 else "b"
                            _add_dep_helper(r.ins, inv_w[wtag].ins, sync=True,
                                            reason="inv scratch RAW")
                            pinv = ps_n.tile([P, QN], f32, tag="pinv",
                                             name="pinv")
                            nc.tensor.matmul(pinv, sel, s2,
                                             start=True, stop=True)
                            nc.vector.tensor_mul(
                                attn_outT[:, j, qc * QN:(qc + 1) * QN],
                                attn_outT[:, j, qc * QN:(qc + 1) * QN],
                                pinv)

                # ---- phase 2: attention, head by head ----
                with (
                    tc.tile_pool(name="ps_o", bufs=1, space="PSUM") as ps_o,
                    tc.tile_pool(name="ps_s", bufs=3, space="PSUM") as ps_s,
                ):
                    for h in range(NH):
                        if h + 2 < NH:
                            load_bias(h + 2)
                        pt = pt_pool.tile([P, KT, N], f16, tag="pt", name="pt")
                        for kt in range(KT):
                            pss = ps_s.tile([P, N], f32, tag="pss", name="pss")
                            for qc in range(QC):
                                nc.tensor.matmul(
                                    pss[:, qc * QN:(qc + 1) * QN],
                                    k_pad[:, h, kt * P:(kt + 1) * P],
                                    q_t[:, h // 2, qc * QN:(qc + 1) * QN],
                                    start=True, stop=True)
                            # P^T = exp(S^T) * exp(bias)  (bias folded
                            # multiplicatively; expB is host-precomputed fp16)
                            nc.scalar.activation(
                                pt[:, kt, :], pss, AF.Exp, bias=0.0, scale=1.0)
                            nc.vector.tensor_mul(
                                pt[:, kt, :], pt[:, kt, :], biasT[h][:, kt, :])
                        pos = [ps_o.tile([HD + 1, QN], f32, tag=f"po{qc}",
                                         name=f"po{qc}") for qc in range(QC)]
                        for kt in range(KT):
                            for qc in range(QC):
                                nc.tensor.matmul(
                                    pos[qc], v_aug[:, kt, h, :],
                                    pt[:, kt, qc * QN:(qc + 1) * QN],
                                    start=(kt == 0), stop=(kt == KT - 1))
                        pbase = (h % 2) * 64
                        for qc in range(QC):
                            po = pos[qc]
                            # unnormalized out2T -> attn_outT rows of the head
                            nc.vector.tensor_copy(
                                attn_outT[pbase:pbase + HD, h // 2,
                                          qc * QN:(qc + 1) * QN],
                                po[0:HD, :])
                            # sums row -> DRAM scratch
                            srow = sums_pool.tile([65, QN], f32, tag="srow",
                                                  name="srow")
                            nc.vector.tensor_copy(srow[64:65, :],
                                                  po[64:65, :])
                            srow_dmas.append(nc.sync.dma_start(
                                sums_d[2 * h + qc], srow[64:65, :]))
                        if h == 9:
                            norm_prep_dve(0, 20, "a")

                # ---- normalization: inverse + PE selector-broadcast ----
                with (
                    tc.tile_pool(name="ps_n", bufs=2, space="PSUM") as ps_n,
                    tc.tile_pool(name="ps_y", bufs=3, space="PSUM") as ps_y,
                ):
                    norm_apply([0, 1, 2, 3, 4], ps_n)
                    norm_prep_act(20, 24, "b")
                    norm_apply([5], ps_n)

                    # ---- phase 3: y^T = wproj^T @ attn_outT ----
                    for j in range(CT):
                        pys = [ps_y.tile([P, QN], f32, tag=f"py{qc}",
                                         name=f"py{qc}") for qc in range(QC)]
                        for k in range(CT):
                            for qc in range(QC):
                                nc.tensor.matmul(
                                    pys[qc], wproj[:, k, j * P:(j + 1) * P],
                                    attn_outT[:, k, qc * QN:(qc + 1) * QN],
                                    start=(k == 0), stop=(k == CT - 1))
                        for qc in range(QC):
                            yb = p2.tile([P, QN], f32, tag="yb", bufs=2,
                                         name="yb")
                            nc.scalar.activation(
                                yb, pys[qc], AF.Identity,
                                bias=pbias[:, j:j + 1], scale=1.0)
                            nc.sync.dma_start(
                                yT_d[:].rearrange("(j p) t -> p j t", p=P)
                                [:, j, qc * QN:(qc + 1) * QN], yb)

    nc.compile()
    return nc


def _get_nc():
    if "nc" not in _CACHE:
        _CACHE["nc"] = _build()
    return _CACHE["nc"]


def prepare_inputs(x, qkv_w, q_bias, v_bias, proj_w, proj_b, rel_table,
                   rel_index):
    """Host-side resharding/layout prep. Returns per-core input maps."""
    scale = HD ** -0.5
    x = np.asarray(x, np.float32)
    qkv_w = np.asarray(qkv_w, np.float32)
    q_bias = np.asarray(q_bias, np.float32)
    v_bias = np.asarray(v_bias, np.float32)
    proj_w = np.asarray(proj_w, np.float32)
    proj_b = np.asarray(proj_b, np.float32)
    rel_table = np.asarray(rel_table, np.float32)
    rel_index = np.asarray(rel_index)

    wq = qkv_w[0:C, :] * scale          # [o, c] rows scaled
    wk = qkv_w[C:2 * C, :]
    wv = qkv_w[2 * C:3 * C, :]
    wqk = np.ascontiguousarray(np.concatenate([wq, wk], axis=0).T)  # [c, 2C]
    wv_t = np.ascontiguousarray(wv.T)                                # [c, C]
    wproj = np.ascontiguousarray(proj_w.T)                           # [c, co]
    qkb = np.concatenate([q_bias * scale, np.zeros(C, np.float32)])
    qkb = np.ascontiguousarray(qkb.reshape(OT_QK, P))
    pb = np.ascontiguousarray(proj_b.reshape(CT, P))

    # bias[q, k, h] = rel_table[rel_index[q, k]]; we ship exp(biasT[h, k, q])
    # so the kernel can fold the softmax bias multiplicatively into P^T
    bias = rel_table[rel_index.reshape(-1)].reshape(N, N, NH)
    biasT = np.ascontiguousarray(
        np.exp(bias.transpose(2, 1, 0), dtype=np.float32)).astype(np.float16)

    sel = np.zeros((2, P), np.float32)
    sel[0, 0:64] = 1.0
    sel[1, 64:128] = 1.0
    shared = {
        "wqk": wqk, "wv": wv_t, "wproj": wproj, "qkb": qkb,
        "vb": v_bias, "pb": pb, "biasT": biasT, "sel": sel,
    }
    in_maps = []
    for b in range(B):
        xt = np.ascontiguousarray(x[b].reshape(N, C).T)
        in_maps.append({"xT": xt, **shared})
    return in_maps


def kernel(x, qkv_w, q_bias, v_bias, proj_w, proj_b, rel_table, rel_index,
           _trace=False):
    from concourse.bass_utils import run_bass_kernel_spmd

    nc = _get_nc()
    in_maps = prepare_inputs(x, qkv_w, q_bias, v_bias, proj_w, proj_b,
                             rel_table, rel_index)
    kwargs = {}
    if _trace:
        import concourse.bass_utils as _bu
        _bu.upload_artifacts = lambda tmpdir: tmpdir
        kwargs = {"trace": True}
    res = run_bass_kernel_spmd(nc, in_maps, core_ids=list(range(B)), **kwargs)
    out = np.empty((B, WS, WS, C), np.float32)
    for b in range(B):
        out[b] = res.results[b]["yT"].T.reshape(WS, WS, C)
    if _trace:
        _CACHE["last_result"] = res
    return out

